# revision 9
# baseline (speedup 1.0000x reference)
"""Trainium2 Bass kernel for nn_Group_Attention (B=2, C=256, N=4096) on 8 NeuronCores.

Sharding: data-parallel over batch B (4 cores per sample); within a sample the
N (query-node) dimension is sharded 4-ways (1024 rows per core).

Structure (v2):
- Phase 0: feat = 0.25*W@x (channel-major), v = elu(4*feat)+1 computed with
  big elementwise ops, then transposed to key-major bf16 via DMA transposes.
  Conv inputs Q_r = W_r@x + b_r, Q_1 = W_1@x + b_1 and their stats also here.
- Attention: software-pipelined loop over 32 key tiles; logits matmul runs one
  iteration ahead of the attn@v matmuls so the PE never waits on the
  vector/scalar/pool exp chain. edge_attr streams as bf16.
- Post: conv(IN(val)+x) is expanded linearly (P = W@val, Q = W@x) so all
  BatchNorm/InstanceNorm statistics for conv_r and conv_1 reduce in a single
  AllReduce together with val's IN stats; a second AllReduce covers conv_2.
  The conv_r affine math and x1 are computed during the second AR's flight.
"""

import numpy as np
import ml_dtypes

import concourse.bass as bass  # noqa: F401
import concourse.tile as tile
import concourse.mybir as mybir
from concourse import bacc
from concourse.bass_isa import ReduceOp
from concourse.bass_utils import run_bass_kernel_spmd

f32 = mybir.dt.float32
f32r = mybir.dt.float32r
bf16 = mybir.dt.bfloat16
u8 = mybir.dt.uint8
AF = mybir.ActivationFunctionType
OP = mybir.AluOpType
AX = mybir.AxisListType

B, C, N = 2, 256, 4096
NCORES = 8
CPB = NCORES // B          # cores per batch sample
R = N // CPB               # query rows per core (1024)
NH = 2                     # n-halves per core
NF = R // NH               # 512 free-dim per chunk
MT = N // 128              # 32 m-tiles (key dim)
CT = C // 128              # 2 channel tiles
EPS = 1e-5
NC_CNT = float(N * C)      # val instance-norm element count per sample

_CACHED_NC = None


def build_nc():
    nc = bacc.Bacc("TRN2", target_bir_lowering=False, debug=False, num_devices=NCORES)

    # ---- per-core I/O ----
    x_d = nc.dram_tensor("x", [128, CT, N], f32r, kind="ExternalInput")
    xs_d = nc.dram_tensor("xs", [128, CT, R], f32r, kind="ExternalInput")
    wT_d = nc.dram_tensor("wT", [128, CT, C], f32r, kind="ExternalInput")
    wrT_d = nc.dram_tensor("wrT", [128, CT, C], f32r, kind="ExternalInput")
    w1T_d = nc.dram_tensor("w1T", [128, CT, C], f32r, kind="ExternalInput")
    w2T_d = nc.dram_tensor("w2T", [128, CT, C], f32r, kind="ExternalInput")
    eaT_d = nc.dram_tensor("eaT", [N, R], bf16, kind="ExternalInput")
    mkT_d = nc.dram_tensor("mkT", [N, R], u8, kind="ExternalInput")
    # packed per-channel params [128, CT, 11]:
    # 0:b_r 1:bn_r_w 2:bn_r_b 3:b1 4:bn1_w 5:bn1_b 6:b2 7:bn2_w 8:bn2_b
    # 9:s_r(=rowsum W_r) 10:s_1(=rowsum W_1)
    prm_d = nc.dram_tensor("prm", [128, CT, 11], f32, kind="ExternalInput")
    bsel_d = nc.dram_tensor("bsel", [1, 2], f32, kind="ExternalInput")
    y_d = nc.dram_tensor("y", [128, CT, R], f32, kind="ExternalOutput")

    with tile.TileContext(nc) as tc:
        with (
            tc.tile_pool(name="const", bufs=1) as const,
            tc.tile_pool(name="st", bufs=1) as st,
            tc.tile_pool(name="small", bufs=1) as small,
            tc.tile_pool(name="psv", bufs=1, space="PSUM") as psv,
            tc.tile_pool(name="dram", bufs=1, space="DRAM") as dram,
        ):
            # ---------------- constants ----------------
            wT = const.tile([128, CT, C], f32r)
            wrT = const.tile([128, CT, C], f32r)
            w1T = const.tile([128, CT, C], f32r)
            w2T = const.tile([128, CT, C], f32r)
            prm = const.tile([128, CT, 11], f32)
            xs = const.tile([128, CT, R], f32r)
            bsel = small.tile([1, 2], f32)
            nc.sync.dma_start(wT[:], wT_d[:])
            nc.sync.dma_start(wrT[:], wrT_d[:])
            nc.sync.dma_start(w1T[:], w1T_d[:])
            nc.sync.dma_start(w2T[:], w2T_d[:])
            nc.sync.dma_start(prm[:], prm_d[:])
            nc.sync.dma_start(bsel[:], bsel_d[:])
            nc.sync.dma_start(xs[:], xs_d[:])
            bselb = small.tile([128, 2], f32)
            nc.gpsimd.partition_broadcast(bselb[:], bsel[:])
            epsb = small.tile([128, 1], f32)
            nc.vector.memset(epsb[:], EPS)
            ones_bf = small.tile([128, 1], bf16)
            nc.vector.memset(ones_bf[:], 1.0)

            feat = const.tile([128, CT, N], f32r, name="feat")     # q/4, chan-major
            featn = const.tile([128, CT, R], f32r, name="featn")   # my rows' q/4
            vcm = const.tile([128, CT, N], bf16, name="vcm")       # elu(q)+1 chan-major
            vT = const.tile([128, MT, C], bf16, name="vT")         # key-major
            valT = const.tile([128, CT, R], f32r, name="valT")
            Qr = const.tile([128, CT, R], f32, name="Qr")
            Q1 = const.tile([128, CT, R], f32, name="Q1")
            sq_scr = const.tile([128, R], f32, name="sq_scr")      # Act Square scratch
            gp_scr = const.tile([128, R], f32, name="gp_scr")      # Pool mult scratch

            ar1_in = const.tile([128, CT, 24], f32, name="ar1_in")
            nc.vector.memset(ar1_in[:], 0.0)

            rawr = small.tile([128, CT, 5], f32)   # conv_r {SP,SP2,SPQ,SQ,SQ2}
            raw1 = small.tile([128, CT, 5], f32)   # conv_1 same

            # ---------------- phase 0a: featn from xs ----------------
            for ct in range(CT):
                for nh in range(NH):
                    fp = psv.tile([128, NF], f32, tag="mm", bufs=2, name="fp")
                    for ci in range(CT):
                        nc.tensor.matmul(
                            fp[:],
                            wT[:, ci, ct * 128:(ct + 1) * 128],
                            xs[:, ci, nh * NF:(nh + 1) * NF],
                            start=(ci == 0), stop=(ci == CT - 1),
                        )
                    if (ct * NH + nh) % 2 == 0:
                        nc.vector.tensor_copy(featn[:, ct, nh * NF:(nh + 1) * NF], fp[:])
                    else:
                        nc.scalar.copy(featn[:, ct, nh * NF:(nh + 1) * NF], fp[:])

            # ---------------- phase 0b: feat from x (4 streamed chunks) ------
            XC = 8
            MX = N // XC            # 512
            with tc.tile_pool(name="xpool", bufs=1) as xpool:
                mv = 0
                for xc in range(XC):
                    x_sb = xpool.tile([128, CT, MX], f32r, tag="xc", bufs=2, name="x_sb")
                    nc.sync.dma_start(x_sb[:], x_d[:, :, xc * MX:(xc + 1) * MX])
                    for ct in range(CT):
                        for sub in range(MX // NF):
                            mbase = xc * MX + sub * NF
                            fp2 = psv.tile([128, NF], f32, tag="mm", bufs=2, name="fp2")
                            for ci in range(CT):
                                nc.tensor.matmul(
                                    fp2[:],
                                    wT[:, ci, ct * 128:(ct + 1) * 128],
                                    x_sb[:, ci, sub * NF:(sub + 1) * NF],
                                    start=(ci == 0), stop=(ci == CT - 1),
                                )
                            dst = feat[:, ct, mbase:mbase + NF]
                            if mv % 2 == 0:
                                nc.vector.tensor_copy(dst, fp2[:])
                            else:
                                nc.scalar.copy(dst, fp2[:])
                            mv += 1

                # ------------ phase 0c: v = elu(4*feat)+1, chan-major -------
                # elu(4f)+1 == min(exp(4f), 1) + max(4f, 0)
                EC = 1024
                for ct in range(CT):
                    for sub in range(N // EC):
                        fsl = feat[:, ct, sub * EC:(sub + 1) * EC].bitcast(f32)
                        eexp = xpool.tile([128, EC], f32, tag="eexp", bufs=2, name="eexp")
                        nc.scalar.activation(eexp[:], fsl, AF.Exp, scale=4.0)
                        erel = xpool.tile([128, EC], f32, tag="erel", bufs=2, name="erel")
                        nc.gpsimd.tensor_scalar(erel[:], fsl, 4.0, 0.0, OP.mult, OP.max)
                        nc.vector.tensor_scalar_min(eexp[:], eexp[:], 1.0)
                        nc.vector.tensor_tensor(
                            vcm[:, ct, sub * EC:(sub + 1) * EC], eexp[:], erel[:], OP.add
                        )

            # ---------------- phase 0d: vT = transpose(vcm) via DMA ---------
            for mt in range(MT):
                for ct in range(CT):
                    nc.scalar.dma_start_transpose(
                        vT[:, mt, ct * 128:(ct + 1) * 128],
                        vcm[:, ct, mt * 128:(mt + 1) * 128],
                    )

            # ---------------- phase 0e: Q convs + their stats ---------------
            for cv, (w_sb, Qt, bslot) in enumerate([(wrT, Qr, 0), (w1T, Q1, 3)]):
                for ct in range(CT):
                    for nh in range(NH):
                        cp = psv.tile([128, NF], f32, tag="mm", bufs=2, name="cp")
                        for ci in range(CT):
                            nc.tensor.matmul(
                                cp[:],
                                w_sb[:, ci, ct * 128:(ct + 1) * 128],
                                xs[:, ci, nh * NF:(nh + 1) * NF],
                                start=(ci == 0), stop=(ci == CT - 1),
                            )
                        dst = Qt[:, ct, nh * NF:(nh + 1) * NF]
                        if cv == 0:
                            nc.scalar.activation(
                                dst, cp[:], AF.Identity, bias=prm[:, ct, bslot:bslot + 1]
                            )
                        else:
                            nc.vector.tensor_scalar_add(dst, cp[:], prm[:, ct, bslot:bslot + 1])
            for ct in range(CT):
                nc.vector.reduce_sum(rawr[:, ct, 3:4], Qr[:, ct, :], axis=AX.X)
                nc.scalar.activation(sq_scr[:], Qr[:, ct, :], AF.Square,
                                     accum_out=rawr[:, ct, 4:5])
                nc.vector.reduce_sum(raw1[:, ct, 3:4], Q1[:, ct, :], axis=AX.X)
                nc.gpsimd.tensor_tensor(gp_scr[:], Q1[:, ct, :], Q1[:, ct, :], OP.mult)
                nc.vector.reduce_sum(raw1[:, ct, 4:5], gp_scr[:], axis=AX.X)

            # ---------------- phase 1: attention (software-pipelined) -------
            vp = [
                [psv.tile([128, NF], f32, tag=f"vp{cb}{nh}", name=f"vp{cb}{nh}")
                 for nh in range(NH)]
                for cb in range(CT)
            ]
            vpr = [psv.tile([1, NF], f32, tag=f"vpr{nh}", name=f"vpr{nh}")
                   for nh in range(NH)]

            et_live = {}

            def vp_block(j):
                for nh in range(NH):
                    esl = et_live.pop((j, nh))
                    for cb in range(CT):
                        nc.tensor.matmul(
                            vp[cb][nh][:], vT[:, j, cb * 128:(cb + 1) * 128], esl[:],
                            start=(j == 0), stop=(j == MT - 1),
                        )
                    nc.tensor.matmul(
                        vpr[nh][:], ones_bf[:], esl[:],
                        start=(j == 0), stop=(j == MT - 1),
                    )

            for i in range(MT):
                ea_t = st.tile([128, R], bf16, tag="ea", bufs=3, name="ea_t")
                nc.sync.dma_start(ea_t[:], eaT_d[i * 128:(i + 1) * 128, :])
                mk_t = st.tile([128, R], u8, tag="mk", bufs=3, name="mk_t")
                nc.gpsimd.dma_start(mk_t[:], mkT_d[i * 128:(i + 1) * 128, :])
                for nh in range(NH):
                    lp = psv.tile([128, NF], f32, tag="mm", bufs=2, name="lp")
                    for ci in range(CT):
                        nc.tensor.matmul(
                            lp[:],
                            feat[:, ci, i * 128:(i + 1) * 128],
                            featn[:, ci, nh * NF:(nh + 1) * NF],
                            start=(ci == 0), stop=(ci == CT - 1),
                        )
                    t_t = st.tile([128, NF], f32, tag=f"t{nh}", bufs=2, name="t_t")
                    nc.vector.tensor_tensor(t_t[:], lp[:], ea_t[:, nh * NF:(nh + 1) * NF],
                                            OP.mult)
                    e_t = st.tile([128, NF], f32, tag=f"e{nh}", bufs=2, name="e_t")
                    nc.scalar.activation(e_t[:], t_t[:], AF.Exp)
                    et = st.tile([128, NF], bf16, tag=f"et{nh}", bufs=2, name="et")
                    nc.gpsimd.tensor_tensor(et[:], e_t[:], mk_t[:, nh * NF:(nh + 1) * NF],
                                            OP.mult)
                    et_live[(i, nh)] = et
                if i >= 1:
                    vp_block(i - 1)
            vp_block(MT - 1)

            # ---------------- phase 2: val = vp/rowsum + IN stats -----------
            for nh in range(NH):
                rs = small.tile([1, NF], f32, tag=f"rs{nh}", name=f"rs{nh}")
                nc.vector.reciprocal(rs[:], vpr[nh][:])
                rsb = small.tile([128, NF], f32, tag=f"rsb{nh}", name=f"rsb{nh}")
                nc.gpsimd.partition_broadcast(rsb[:], rs[:])
                for cb in range(CT):
                    nc.vector.tensor_tensor(valT[:, cb, nh * NF:(nh + 1) * NF],
                                            vp[cb][nh][:], rsb[:], OP.mult)
            valf = valT[:].bitcast(f32)
            stv = small.tile([128, 2], f32)
            stq2 = small.tile([128, CT], f32)
            nc.vector.reduce_sum(stv[:, 0:1], valf, axis=AX.XY)
            for ct in range(CT):
                nc.scalar.activation(sq_scr[:], valf[:, ct, :], AF.Square,
                                     accum_out=stq2[:, ct:ct + 1])
            nc.vector.tensor_tensor(stv[:, 1:2], stq2[:, 0:1], stq2[:, 1:2], OP.add)
            stvr = small.tile([128, 2], f32)
            nc.gpsimd.partition_all_reduce(stvr[:], stv[:], 128, ReduceOp.add)
            for b in range(B):
                eng = nc.vector if b == 0 else nc.gpsimd
                eng.tensor_scalar_mul(ar1_in[:, 0, 2 * b:2 * b + 2], stvr[:],
                                      bselb[:, b:b + 1])

            # ---------------- phase 3: P convs + stats ----------------------
            P = const.tile([128, 2, CT, R], f32, tag="feat", name="P")
            mv = 0
            for cv, w_sb in enumerate([wrT, w1T]):
                for ct in range(CT):
                    for nh in range(NH):
                        pp = psv.tile([128, NF], f32, tag="mm", bufs=2, name="pp")
                        for ci in range(CT):
                            nc.tensor.matmul(
                                pp[:],
                                w_sb[:, ci, ct * 128:(ct + 1) * 128],
                                valT[:, ci, nh * NF:(nh + 1) * NF],
                                start=(ci == 0), stop=(ci == CT - 1),
                            )
                        dst = P[:, cv, ct, nh * NF:(nh + 1) * NF]
                        if mv % 2 == 0:
                            nc.vector.tensor_copy(dst, pp[:])
                        else:
                            nc.scalar.copy(dst, pp[:])
                        mv += 1
            raws = [rawr, raw1]
            for cv in range(2):
                Qt = [Qr, Q1][cv]
                for ct in range(CT):
                    Psl = P[:, cv, ct, :]
                    nc.vector.reduce_sum(raws[cv][:, ct, 0:1], Psl, axis=AX.X)
                    nc.scalar.activation(sq_scr[:], Psl, AF.Square,
                                         accum_out=raws[cv][:, ct, 1:2])
                    nc.gpsimd.tensor_tensor(gp_scr[:], Psl, Qt[:, ct, :], OP.mult)
                    nc.vector.reduce_sum(raws[cv][:, ct, 2:3], gp_scr[:], axis=AX.X)
            for cv in range(2):
                for ct in range(CT):
                    for b in range(B):
                        base = 4 + 10 * cv + 5 * b
                        eng = nc.vector if b == 0 else nc.gpsimd
                        eng.tensor_scalar_mul(ar1_in[:, ct, base:base + 5],
                                              raws[cv][:, ct, 0:5], bselb[:, b:b + 1])

            # ---------------- AR1: everything except conv2 stats ------------
            ar1_ind = dram.tile([128, CT, 24], f32)
            ar1_outd = dram.tile([128, CT, 24], f32)
            nc.sync.dma_start(ar1_ind[:], ar1_in[:])
            nc.gpsimd.collective_compute(
                "AllReduce", OP.add, replica_groups=[list(range(NCORES))],
                ins=[ar1_ind.opt()], outs=[ar1_outd.opt()],
            )
            ar1 = const.tile([128, CT, 24], f32, name="ar1")
            nc.sync.dma_start(ar1[:], ar1_outd[:])

            # ---------------- phase 4: post-AR1 scalar math -----------------
            uid = [0]

            def tl(shape=(128, 1)):
                uid[0] += 1
                t = small.tile(list(shape), f32, name=f"aff{uid[0]}")
                return t

            # per-batch val IN scalars
            mu_, rstd_, m_, r2_, mN_, twom_, Nm2_, tworstd_ = ({} for _ in range(8))
            for b in range(B):
                ve = nc.vector if b == 0 else nc.gpsimd
                sl = ar1[:, 0, 2 * b:2 * b + 1]
                sq = ar1[:, 0, 2 * b + 1:2 * b + 2]
                mu = tl(); ve.tensor_scalar_mul(mu[:], sl, 1.0 / NC_CNT)
                e2 = tl(); ve.tensor_scalar_mul(e2[:], sq, 1.0 / NC_CNT)
                mu2 = tl(); ve.tensor_tensor(mu2[:], mu[:], mu[:], OP.mult)
                var = tl(); ve.tensor_tensor(var[:], e2[:], mu2[:], OP.subtract)
                sd = tl(); nc.scalar.activation(sd[:], var[:], AF.Sqrt, bias=epsb[:])
                rstd = tl(); nc.vector.reciprocal(rstd[:], sd[:])
                m = tl(); ve.tensor_tensor(m[:], mu[:], rstd[:], OP.mult)
                r2 = tl(); ve.tensor_tensor(r2[:], rstd[:], rstd[:], OP.mult)
                mN = tl(); ve.tensor_scalar_mul(mN[:], m[:], float(N))
                twom = tl(); ve.tensor_scalar_mul(twom[:], m[:], 2.0)
                m2s = tl(); ve.tensor_tensor(m2s[:], m[:], m[:], OP.mult)
                Nm2 = tl(); ve.tensor_scalar_mul(Nm2[:], m2s[:], float(N))
                twor = tl(); ve.tensor_scalar_mul(twor[:], rstd[:], 2.0)
                mu_[b], rstd_[b], m_[b], r2_[b] = mu, rstd, m, r2
                mN_[b], twom_[b], Nm2_[b], tworstd_[b] = mN, twom, Nm2, twor

            def bselect(a0, a1, eng, shape=(128, 1)):
                t0 = tl(shape); eng.tensor_scalar_mul(t0[:], a0, bselb[:, 0:1])
                t1 = tl(shape); eng.tensor_scalar_mul(t1[:], a1, bselb[:, 1:2])
                o = tl(shape); eng.tensor_tensor(o[:], t0[:], t1[:], OP.add)
                return o

            rstd_my = bselect(rstd_[0][:], rstd_[1][:], nc.vector)
            m_my = bselect(m_[0][:], m_[1][:], nc.gpsimd)

            SH = (128, CT, 1)

            def analytic_moments(slots_base, s_ap, s2_ap):
                """Per-batch (mean, var) over N of o = rstd_b*P + Q - m_b*s."""
                m1l, v1l = [], []
                for b in range(B):
                    eng = nc.vector if b == 0 else nc.gpsimd
                    base = slots_base + 5 * b
                    SP = ar1[:, :, base:base + 1]
                    SP2 = ar1[:, :, base + 1:base + 2]
                    SPQ = ar1[:, :, base + 2:base + 3]
                    SQ = ar1[:, :, base + 3:base + 4]
                    SQ2 = ar1[:, :, base + 4:base + 5]
                    tsum = tl(SH); eng.tensor_scalar_mul(tsum[:], SP, rstd_[b][:])
                    eng.tensor_tensor(tsum[:], tsum[:], SQ, OP.add)
                    u = tl(SH); eng.tensor_scalar_mul(u[:], s_ap, mN_[b][:])
                    T1 = tl(SH); eng.tensor_tensor(T1[:], tsum[:], u[:], OP.subtract)
                    a = tl(SH); eng.tensor_scalar_mul(a[:], SP2, r2_[b][:])
                    c = tl(SH); eng.tensor_scalar_mul(c[:], SPQ, tworstd_[b][:])
                    eng.tensor_tensor(a[:], a[:], c[:], OP.add)
                    eng.tensor_tensor(a[:], a[:], SQ2, OP.add)
                    d = tl(SH); eng.tensor_tensor(d[:], s_ap, tsum[:], OP.mult)
                    eng.tensor_scalar_mul(d[:], d[:], twom_[b][:])
                    eng.tensor_tensor(a[:], a[:], d[:], OP.subtract)
                    f = tl(SH); eng.tensor_scalar_mul(f[:], s2_ap, Nm2_[b][:])
                    eng.tensor_tensor(a[:], a[:], f[:], OP.add)
                    m1 = tl(SH); eng.tensor_scalar_mul(m1[:], T1[:], 1.0 / N)
                    e21 = tl(SH); eng.tensor_scalar_mul(e21[:], a[:], 1.0 / N)
                    m1sq = tl(SH); eng.tensor_tensor(m1sq[:], m1[:], m1[:], OP.mult)
                    v1 = tl(SH); eng.tensor_tensor(v1[:], e21[:], m1sq[:], OP.subtract)
                    m1l.append(m1)
                    v1l.append(v1)
                return m1l, v1l

            def finish_in_bn(m1l, v1l, w_ap, b_ap):
                """Composed affine of BN(IN(o)) given per-(b,c) moments of o."""
                rats = []
                for b in range(B):
                    eng = nc.vector if b == 0 else nc.gpsimd
                    vpe = tl(SH); eng.tensor_scalar_add(vpe[:], v1l[b][:], EPS)
                    rv = tl(SH); nc.vector.reciprocal(rv[:], vpe[:])
                    rat = tl(SH); eng.tensor_tensor(rat[:], v1l[b][:], rv[:], OP.mult)
                    rats.append(rat)
                varbn = tl(SH)
                nc.vector.tensor_tensor(varbn[:], rats[0][:], rats[1][:], OP.add)
                nc.vector.tensor_scalar_mul(varbn[:], varbn[:], 0.5)
                sdbn = tl(SH); nc.scalar.activation(sdbn[:], varbn[:], AF.Sqrt, bias=epsb[:])
                rstdbn = tl(SH); nc.vector.reciprocal(rstdbn[:], sdbn[:])
                m1my = bselect(m1l[0][:], m1l[1][:], nc.gpsimd, SH)
                v1my = bselect(v1l[0][:], v1l[1][:], nc.vector, SH)
                sdmy = tl(SH); nc.scalar.activation(sdmy[:], v1my[:], AF.Sqrt, bias=epsb[:])
                rstd1my = tl(SH); nc.vector.reciprocal(rstd1my[:], sdmy[:])
                al = tl(SH)
                nc.vector.tensor_tensor(al[:], rstd1my[:], rstdbn[:], OP.mult)
                nc.vector.tensor_tensor(al[:], al[:], w_ap, OP.mult)
                be = tl(SH)
                nc.gpsimd.tensor_tensor(be[:], m1my[:], al[:], OP.mult)
                nc.gpsimd.tensor_tensor(be[:], b_ap, be[:], OP.subtract)
                return al, be

            s1_ap = prm[:, :, 10:11]
            s1sq = tl(SH); nc.vector.tensor_tensor(s1sq[:], s1_ap, s1_ap, OP.mult)
            m1l_1, v1l_1 = analytic_moments(14, s1_ap, s1sq[:])
            al1, be1 = finish_in_bn(m1l_1, v1l_1, prm[:, :, 4:5], prm[:, :, 5:6])
            # h = relu(A1*P1 + al1*Q1 + C1)
            A1 = tl(SH); nc.vector.tensor_scalar_mul(A1[:], al1[:], rstd_my[:])
            C1 = tl(SH)
            nc.gpsimd.tensor_scalar_mul(C1[:], s1_ap, m_my[:])
            nc.gpsimd.tensor_tensor(C1[:], C1[:], al1[:], OP.mult)
            nc.gpsimd.tensor_tensor(C1[:], be1[:], C1[:], OP.subtract)

            h = const.tile([128, CT, R], f32r, tag="vcm", name="h")
            for ct in range(CT):
                th = st.tile([128, R], f32, tag="big0", bufs=1, name="th")
                nc.vector.tensor_scalar_mul(th[:], P[:, 1, ct, :], A1[:, ct, 0:1])
                uh = st.tile([128, R], f32, tag="big1", bufs=1, name="uh")
                nc.gpsimd.tensor_scalar(uh[:], Q1[:, ct, :], al1[:, ct, 0:1],
                                        C1[:, ct, 0:1], OP.mult, OP.add)
                wh = st.tile([128, R], f32, tag="big2", bufs=1, name="wh")
                nc.vector.tensor_tensor(wh[:], th[:], uh[:], OP.add)
                nc.scalar.activation(h[:, ct, :], wh[:], AF.Relu)

            # ---------------- phase 5: conv2 + stats + AR2 ------------------
            o2 = const.tile([128, CT, R], f32, tag="xs", name="o2")
            for ct in range(CT):
                for nh in range(NH):
                    cp2 = psv.tile([128, NF], f32, tag="mm", bufs=2, name="cp2")
                    for ci in range(CT):
                        nc.tensor.matmul(
                            cp2[:],
                            w2T[:, ci, ct * 128:(ct + 1) * 128],
                            h[:, ci, nh * NF:(nh + 1) * NF],
                            start=(ci == 0), stop=(ci == CT - 1),
                        )
                    nc.scalar.activation(o2[:, ct, nh * NF:(nh + 1) * NF], cp2[:],
                                         AF.Identity, bias=prm[:, ct, 6:7])
            ro2 = small.tile([128, CT, 2], f32)
            for ct in range(CT):
                nc.vector.reduce_sum(ro2[:, ct, 0:1], o2[:, ct, :], axis=AX.X)
                nc.scalar.activation(sq_scr[:], o2[:, ct, :], AF.Square,
                                     accum_out=ro2[:, ct, 1:2])
            ar2_in = small.tile([128, CT, 4], f32)
            for b in range(B):
                eng = nc.vector if b == 0 else nc.gpsimd
                eng.tensor_scalar_mul(ar2_in[:, :, 2 * b:2 * b + 2], ro2[:, :, 0:2],
                                      bselb[:, b:b + 1])
            ar2_ind = dram.tile([128, CT, 4], f32)
            ar2_outd = dram.tile([128, CT, 4], f32)
            nc.sync.dma_start(ar2_ind[:], ar2_in[:])
            nc.gpsimd.collective_compute(
                "AllReduce", OP.add, replica_groups=[list(range(NCORES))],
                ins=[ar2_ind.opt()], outs=[ar2_outd.opt()],
            )
            ar2 = small.tile([128, CT, 4], f32)
            nc.sync.dma_start(ar2[:], ar2_outd[:])

            # -------- during AR2 flight: conv_r affine + x1 -----------------
            sr_ap = prm[:, :, 9:10]
            srsq = tl(SH); nc.vector.tensor_tensor(srsq[:], sr_ap, sr_ap, OP.mult)
            m1l_r, v1l_r = analytic_moments(4, sr_ap, srsq[:])
            # bn_r: plain BatchNorm over (B,N): combine both batches' moments
            Sor = tl(SH)
            nc.vector.tensor_tensor(Sor[:], m1l_r[0][:], m1l_r[1][:], OP.add)
            nc.vector.tensor_scalar_mul(Sor[:], Sor[:], 0.5)        # global mean
            # E[o^2] = mean_b (v1_b + m1_b^2)
            e2r = tl(SH)
            t0 = tl(SH); nc.vector.tensor_tensor(t0[:], m1l_r[0][:], m1l_r[0][:], OP.mult)
            t1 = tl(SH); nc.vector.tensor_tensor(t1[:], m1l_r[1][:], m1l_r[1][:], OP.mult)
            nc.vector.tensor_tensor(e2r[:], v1l_r[0][:], t0[:], OP.add)
            nc.vector.tensor_tensor(t1[:], v1l_r[1][:], t1[:], OP.add)
            nc.vector.tensor_tensor(e2r[:], e2r[:], t1[:], OP.add)
            nc.vector.tensor_scalar_mul(e2r[:], e2r[:], 0.5)
            Mr2 = tl(SH); nc.vector.tensor_tensor(Mr2[:], Sor[:], Sor[:], OP.mult)
            Vr = tl(SH); nc.vector.tensor_tensor(Vr[:], e2r[:], Mr2[:], OP.subtract)
            sdr = tl(SH); nc.scalar.activation(sdr[:], Vr[:], AF.Sqrt, bias=epsb[:])
            rstdr = tl(SH); nc.vector.reciprocal(rstdr[:], sdr[:])
            alr = tl(SH); nc.vector.tensor_tensor(alr[:], prm[:, :, 1:2], rstdr[:], OP.mult)
            ber = tl(SH)
            nc.vector.tensor_tensor(ber[:], Sor[:], alr[:], OP.mult)
            nc.vector.tensor_tensor(ber[:], prm[:, :, 2:3], ber[:], OP.subtract)
            # x1 = Ar*P_r + alr*Q_r + Cr
            Ar = tl(SH); nc.vector.tensor_scalar_mul(Ar[:], alr[:], rstd_my[:])
            Cr = tl(SH)
            nc.gpsimd.tensor_scalar_mul(Cr[:], sr_ap, m_my[:])
            nc.gpsimd.tensor_tensor(Cr[:], Cr[:], alr[:], OP.mult)
            nc.gpsimd.tensor_tensor(Cr[:], ber[:], Cr[:], OP.subtract)
            x1 = const.tile([128, CT, R], f32, tag="featn", name="x1")
            for ct in range(CT):
                tx = st.tile([128, R], f32, tag="big0", bufs=1, name="tx")
                nc.vector.tensor_scalar_mul(tx[:], P[:, 0, ct, :], Ar[:, ct, 0:1])
                ux = st.tile([128, R], f32, tag="big1", bufs=1, name="ux")
                nc.gpsimd.tensor_scalar(ux[:], Qr[:, ct, :], alr[:, ct, 0:1],
                                        Cr[:, ct, 0:1], OP.mult, OP.add)
                nc.vector.tensor_tensor(x1[:, ct, :], tx[:], ux[:], OP.add)

            # ---------------- phase 6: post-AR2 bn2 + output ----------------
            m2l, v2l = [], []
            for b in range(B):
                eng = nc.vector if b == 0 else nc.gpsimd
                m2 = tl(SH); eng.tensor_scalar_mul(m2[:], ar2[:, :, 2 * b:2 * b + 1], 1.0 / N)
                e22 = tl(SH); eng.tensor_scalar_mul(e22[:], ar2[:, :, 2 * b + 1:2 * b + 2], 1.0 / N)
                m2sq = tl(SH); eng.tensor_tensor(m2sq[:], m2[:], m2[:], OP.mult)
                v2 = tl(SH); eng.tensor_tensor(v2[:], e22[:], m2sq[:], OP.subtract)
                m2l.append(m2)
                v2l.append(v2)
            al2, be2 = finish_in_bn(m2l, v2l, prm[:, :, 7:8], prm[:, :, 8:9])

            y_sb = const.tile([128, CT, R], f32, tag="Q1", name="y_sb")
            for ct in range(CT):
                ty = st.tile([128, R], f32, tag="big0", bufs=1, name="ty")
                nc.vector.tensor_scalar(ty[:], o2[:, ct, :], al2[:, ct, 0:1],
                                        be2[:, ct, 0:1], OP.mult, OP.add)
                wy = st.tile([128, R], f32, tag="big1", bufs=1, name="wy")
                nc.gpsimd.tensor_tensor(wy[:], ty[:], x1[:, ct, :], OP.add)
                nc.scalar.activation(y_sb[:, ct, :], wy[:], AF.Relu)
                nc.scalar.dma_start(y_d[:, ct, :], y_sb[:, ct, :])

    nc.compile()
    return nc


def _prep_core_inputs(inputs):
    """Build the 8 per-core in_maps from the full problem inputs."""
    x = np.asarray(inputs["x"], dtype=np.float32)          # (B,C,N,1)
    edge_map = np.asarray(inputs["edge_map"])              # (B,N,N) int32
    edge_attr = np.asarray(inputs["edge_attr"], dtype=np.float32)

    def chan_major(w):  # (C, X) -> [128, CT, X]
        return np.ascontiguousarray(
            w.reshape(CT, 128, -1).transpose(1, 0, 2)
        ).astype(np.float32)

    # linear weight pre-scaled by 0.25 so q*q carries the 1/16 temperature
    wT = chan_major(np.asarray(inputs["linear_w"], dtype=np.float32).T * 0.25)
    wrT = chan_major(np.asarray(inputs["w_r"], dtype=np.float32).T)
    w1T = chan_major(np.asarray(inputs["w1"], dtype=np.float32).T)
    w2T = chan_major(np.asarray(inputs["w2"], dtype=np.float32).T)

    s_r = np.asarray(inputs["w_r"], np.float32).sum(axis=1)
    s_1 = np.asarray(inputs["w1"], np.float32).sum(axis=1)
    pvals = [np.asarray(inputs[p], np.float32) for p in
             ["b_r", "bn_r_w", "bn_r_b", "b1", "bn1_w", "bn1_b", "b2", "bn2_w", "bn2_b"]]
    pvals += [s_r, s_1]
    prm = np.stack([p.reshape(CT, 128) for p in pvals], axis=-1).transpose(1, 0, 2)
    prm = np.ascontiguousarray(prm)  # [128, CT, 11]

    in_maps = []
    for core in range(NCORES):
        b = core // CPB
        r0 = (core % CPB) * R
        xb = x[b, :, :, 0]                                  # (C, N)
        x_cm = chan_major(xb)                               # [128, CT, N]
        xs_cm = np.ascontiguousarray(x_cm[:, :, r0:r0 + R])
        eaT = np.ascontiguousarray(edge_attr[b].T[:, r0:r0 + R]).astype(ml_dtypes.bfloat16)
        mkT = np.ascontiguousarray((edge_map[b].T[:, r0:r0 + R] != 0).astype(np.uint8))
        bsel = np.zeros((1, 2), np.float32)
        bsel[0, b] = 1.0
        in_maps.append({
            "x": x_cm, "xs": xs_cm, "wT": wT, "wrT": wrT, "w1T": w1T, "w2T": w2T,
            "eaT": eaT, "mkT": mkT, "prm": prm, "bsel": bsel,
        })
    return in_maps


def run(inputs, trace=False):
    global _CACHED_NC
    if _CACHED_NC is None:
        _CACHED_NC = build_nc()
    nc = _CACHED_NC
    in_maps = _prep_core_inputs(inputs)
    res = run_bass_kernel_spmd(
        nc, in_maps, core_ids=list(range(NCORES)), trace=trace
    )
    out = np.zeros((B, C, N, 1), np.float32)
    for core in range(NCORES):
        b = core // CPB
        r0 = (core % CPB) * R
        shard = res.results[core]["y"]                      # [128, CT, R]
        out[b, :, r0:r0 + R, 0] = shard.transpose(1, 0, 2).reshape(C, R)
    return out, res


def kernel(**inputs) -> np.ndarray:
    out, _ = run(inputs, trace=False)
    return out
